# revision 25
# baseline (speedup 1.0000x reference)
"""Trainium2 Bass kernel for nn_ComplexConv2Deffangle4Dxy.

Reference math (per batch b, branch br):
    out[br] = pointwise(w2, depthwise3x3(w1, img[br]))   with zero padding P=1
      br=0 (rot): weights (w1n, w2n) where wn = (wx+wy)^2 / sum((wx+wy)^2)
      br=1 (abs): log-domain: exp(branch(log(img + EPS), w1n, w2n))
      br=2 (x):   weights (w1x, w2x)
      br=3 (y):   weights (w1y, w2y)

Kernel strategy (per NeuronCore, data-parallel over batch B=8 -> 8 cores):
  Fuse depthwise+pointwise into a single 3x3 conv whose weights are the
  outer product  Wf[o, c, k] = w2[o, c] * w1[c, k], computed as
  PSUM-accumulated matmuls over the 9 kernel offsets with
  lhsT = fused weights (K, M=Cout=128) and rhs = shifted image views.
  Images are zero-padded on the host (pure marshaling) so every shifted
  view is a plain strided AP with no boundary special cases; for the abs
  branch Ln(x*1+EPS) maps the zero padding to log(EPS), exactly matching
  the reference's pad-then-log order.  Weight normalization for the
  rot/abs branches is computed on device (sum via ones-matmul, reciprocal
  on DVE, scale folded into the fused conv weights).

  Scheme "pack6": SBUF partitions 0..63 hold the padded image A,
  partitions 64..127 hold B with B[r] = A[r+2].  A K=128 matmul at row
  offset r contracts tap (dh=-1,dw) on the lower half and (dh=+1,dw) on
  the upper half in one instruction (3 pair matmuls); the three dh=0
  taps are K=128 matmuls whose upper-half weights are zero.  6 matmuls
  per 8-row output tile, all K=128.

  Scheme "pack5": like pack6 plus a second tile img2 = [A; C] with
  C[r, c] = A[r, c+2], derived on-device via SBUF->SBUF DMA, which lets
  taps (0,-1)/(0,+1) share one K=128 matmul.  5 matmuls per tile.

  Legacy schemes "dual" and "hsplit" are kept for comparison (see git
  history / docstrings in _mm_dual/_mm_hsplit).
"""

import sys

for _p in ("/opt/trn_rl_repo",):
    if _p not in sys.path:
        sys.path.insert(0, _p)

import ml_dtypes
import numpy as np

import concourse.bacc as bacc
import concourse.mybir as mybir
import concourse.tile as tile
from concourse import bass_utils

F32 = mybir.dt.float32
F32R = mybir.dt.float32r
BF16 = mybir.dt.bfloat16

EPS = 1e-6
N_CORES = 8
B, NBR, CIN, COUT, H, W = 8, 4, 64, 128, 64, 64
HP, WP = H + 2, W + 2          # host-padded image
HS_ROWS = 35                   # hsplit: padded rows per partition half

# matmul input dtype: "f32r" | "f32" | "bf16"
MM_DTYPE = "f32r"
SCHEME = "pack5"               # "pack5" | "pack6" | "dual" | "hsplit"
OUT_DT = "f32"                 # "f32" | "bf16" (device output; host casts)
ORDER = "kout"                 # pack schemes: "kout" (slot-outer) | "tout"
# Packing (0,+1) onto the upper PE row group (K=64 at base_partition 64)
# mixed with K=128 matmuls in the same PSUM accumulation group crashes at
# runtime on TRN2 hardware -- keep disabled.  (pack5/pack6 use uniform
# K=128 matmuls everywhere and avoid the issue.)
DH0_UPPER_PACK = False
LOOP_ITERS = None              # benchmarking: device-side repeat count
PROBE = ""                     # "" | "no_out" (skip evac+out-DMA) | "no_mm"
                               # | "no_in" (skip input DMA+Ln) | "no_in2" (skip derives)
QCFG = ""                      # DMA queue split: "o"->out on Act DGE queue,
                               # "d"->img2 derives on Act DGE queue ("" = all SP)
TRACE = False
LAST_EXEC_TIME_NS = None
LAST_RESULTS = None

_PROG_CACHE = {}

# walrus's LDWEIGHTS optimization (split weight loads from matmuls so they
# pipeline through the PE reorder window) is hardcoded off in
# bass_utils.bir_verify_and_optimise; expose a switch that rewrites the flag
# inside the compile command.  NOTE: unusable with f32r (standalone
# LDWEIGHTS yields all-zero output for 4-byte dtypes).
LDW_OPT = False
_orig_run_command = bass_utils.run_command


def _patched_run_command(cmd, *a, **kw):
    if LDW_OPT and isinstance(cmd, list) and "--enable-ldw-opt=false" in cmd:
        cmd = ["--enable-ldw-opt=true" if c == "--enable-ldw-opt=false" else c for c in cmd]
    return _orig_run_command(cmd, *a, **kw)


bass_utils.run_command = _patched_run_command
if getattr(bass_utils, "bir_verify_and_optimise", None) is not None:
    bass_utils.bir_verify_and_optimise.__globals__["run_command"] = _patched_run_command

BRANCHES = (  # (branch index, weight set, log-domain?, evac engine)
    (2, "x", False, "v"),
    (3, "y", False, "a"),
    (1, "n", True, "a"),
    (0, "n", False, "v"),
)

# pack schemes: per-half {slot: k} fused-weight maps and slot rhs offsets.
# Slot rhs is (tile2?, row_base_delta, col_base); row index = 8*tp + delta.
PACK_CFG = {
    "pack6": {
        "n_slots": 6,
        "low_ks": {0: 0, 1: 1, 2: 2, 3: 3, 4: 4, 5: 5},
        "up_ks": {0: 6, 1: 7, 2: 8},
        "rhs": {
            0: (0, 0, 0),
            1: (0, 0, 1),
            2: (0, 0, 2),
            3: (0, 1, 0),
            4: (0, 1, 1),
            5: (0, 1, 2),
        },
    },
    "pack5": {
        "n_slots": 5,
        "low_ks": {0: 0, 1: 1, 2: 2, 3: 3, 4: 4},
        "up_ks": {0: 6, 1: 7, 2: 8, 3: 5},
        "rhs": {
            0: (0, 0, 0),
            1: (0, 0, 1),
            2: (0, 0, 2),
            3: (1, 1, 0),
            4: (0, 1, 1),
        },
    },
}
# pack5h: same slots as pack5, but img2 = [A cols 0..63; A cols 2..65] is
# marshaled on the host and DMAed from HBM instead of derived on-device.
PACK_CFG["pack5h"] = PACK_CFG["pack5"]


def _mm_dt():
    return {"f32r": F32R, "f32": F32, "bf16": BF16}[MM_DTYPE]


def _out_dt():
    return BF16 if OUT_DT == "bf16" else F32


def _np_in_dt():
    return ml_dtypes.bfloat16 if MM_DTYPE == "bf16" else np.float32


def _np_out_dt():
    return ml_dtypes.bfloat16 if OUT_DT == "bf16" else np.float32


def _emit(nc, tc, xin_d, xin2_d, w1x_d, w1y_d, w2xT_d, w2yT_d, out_d):
    mdt = _mm_dt()
    odt = _out_dt()
    packed = SCHEME in PACK_CFG
    if SCHEME == "hsplit":
        img_rows = HS_ROWS
    else:
        img_rows = HP
    n_psum = 1 if packed else 6
    with (
        tc.tile_pool(name="wp", bufs=1) as wp,
        tc.tile_pool(name="imgp", bufs=2) as imgp,
        tc.tile_pool(name="img2p", bufs=2) as img2p,
        tc.tile_pool(name="obp", bufs=6) as obp,
    ):
        # ---- weight prep -------------------------------------------------
        # All weight/source tiles replicated into both partition halves so
        # per-half fused tiles can be built with partition-local DVE ops.
        # The reduction PSUM pool lives in its own scope so the main loop
        # can use all 8 PSUM banks.
        w1x_s = wp.tile([2 * CIN, 9], F32, tag="w1x")
        w1y_s = wp.tile([2 * CIN, 9], F32, tag="w1y")
        w2xT_s = wp.tile([2 * CIN, COUT], F32, tag="w2xT")
        w2yT_s = wp.tile([2 * CIN, COUT], F32, tag="w2yT")
        for t, d in (
            (w1x_s, w1x_d),
            (w1y_s, w1y_d),
            (w2xT_s, w2xT_d),
            (w2yT_s, w2yT_d),
        ):
            nc.sync.dma_start(out=t[0:CIN], in_=d)
            nc.sync.dma_start(out=t[CIN : 2 * CIN], in_=d)

        ones_k = wp.tile([CIN, 1], F32, tag="ones_k")
        nc.vector.memset(ones_k[:, :], 1.0)
        ones_m = wp.tile([1, 2 * CIN], F32, tag="ones_m")
        nc.vector.memset(ones_m[:, :], 1.0)
        eps_b = wp.tile([2 * CIN, 1], F32, tag="eps_b")
        nc.vector.memset(eps_b[:, :], float(EPS))
        zero_b = wp.tile([COUT, 1], F32, tag="zero_b")
        nc.vector.memset(zero_b[:, :], 0.0)
        zero_c = wp.tile([2 * CIN, 1], F32, tag="zero_c")
        nc.vector.memset(zero_c[:, :], 0.0)

        # u1 = (w1x + w1y)^2, u2T = ((w2x + w2y)^2)^T  (both partition halves)
        u1 = wp.tile([2 * CIN, 9], F32, tag="u1")
        nc.vector.tensor_add(u1[:, :], w1x_s[:, :], w1y_s[:, :])
        nc.vector.tensor_mul(u1[:, :], u1[:, :], u1[:, :])
        u2T = wp.tile([2 * CIN, COUT], F32, tag="u2T")
        nc.vector.tensor_add(u2T[:, :], w2xT_s[:, :], w2yT_s[:, :])
        nc.vector.tensor_mul(u2T[:, :], u2T[:, :], u2T[:, :])

        u2Tn = wp.tile([2 * CIN, COUT], F32, tag="u2Tn")
        with tc.tile_pool(name="psr", bufs=2, space="PSUM") as psr:
            # S1 = sum(u1), S2 = sum(u2) via ones-matmul + free-dim reduce
            s1v = psr.tile([1, 9], F32, tag="red")
            nc.tensor.matmul(s1v[:, :], ones_k[:, :], u1[0:CIN, :], start=True, stop=True)
            s2v = psr.tile([1, COUT], F32, tag="red")
            nc.tensor.matmul(s2v[:, :], ones_k[:, :], u2T[0:CIN, :], start=True, stop=True)
            s1 = wp.tile([1, 1], F32, tag="s1")
            nc.vector.tensor_reduce(
                s1[:, :], s1v[:, :], axis=mybir.AxisListType.X, op=mybir.AluOpType.add
            )
            s2 = wp.tile([1, 1], F32, tag="s2")
            nc.vector.tensor_reduce(
                s2[:, :], s2v[:, :], axis=mybir.AxisListType.X, op=mybir.AluOpType.add
            )
            inv = wp.tile([1, 1], F32, tag="inv")
            nc.vector.tensor_mul(inv[:, :], s1[:, :], s2[:, :])
            nc.vector.reciprocal(inv[:, :], inv[:, :])
            # broadcast 1/(S1*S2) to all 128 partitions
            invb_ps = psr.tile([2 * CIN, 1], F32, tag="red")
            nc.tensor.matmul(invb_ps[:, :], ones_m[:, :], inv[:, :], start=True, stop=True)
            invb = wp.tile([2 * CIN, 1], F32, tag="invb")
            nc.vector.tensor_copy(invb[:, :], invb_ps[:, :])
            # u2T_n = u2T / (S1*S2): both normalizations in one fold
            nc.vector.tensor_scalar(
                u2Tn[:, :], u2T[:, :], invb[:, 0:1], None, mybir.AluOpType.mult
            )

        # fused weight tiles: per half, slot j holds w2T * w1[:, k] per the
        # scheme's slot->k map; unmapped upper slots are zero.
        if packed:
            cfg = PACK_CFG[SCHEME]
            n_blocks = cfg["n_slots"]
            half_maps = (cfg["low_ks"], cfg["up_ks"])
        elif SCHEME == "dual":
            half_maps = (
                {0: 0, 1: 1, 2: 2, 3: 3, 4: 4, 5: 5},
                {0: 6, 1: 7, 2: 8, 3: 5},
            )
            n_blocks = 6
        else:
            half_maps = (
                {k: k for k in range(9)},
                {k: k for k in range(9)},
            )
            n_blocks = 9
        wf_tiles = {}
        for s, base, w1s in (("x", w2xT_s, w1x_s), ("y", w2yT_s, w1y_s), ("n", u2Tn, u1)):
            wf = wp.tile([2 * CIN, n_blocks * COUT], mdt, tag=f"wf{s}")
            for half in (0, 1):
                p0, p1 = half * CIN, (half + 1) * CIN
                kmap = half_maps[half]
                for slot in range(n_blocks):
                    blk = wf[p0:p1, slot * COUT : (slot + 1) * COUT]
                    if slot in kmap:
                        nc.vector.tensor_scalar(
                            blk,
                            base[p0:p1, :],
                            w1s[p0:p1, kmap[slot] : kmap[slot] + 1],
                            None,
                            mybir.AluOpType.mult,
                        )
                    else:
                        # mult-by-zero instead of memset: walrus rejects
                        # InstMemset on f32r tiles
                        nc.vector.tensor_scalar(
                            blk,
                            base[p0:p1, :],
                            zero_c[p0:p1, 0:1],
                            None,
                            mybir.AluOpType.mult,
                        )
            wf_tiles[s] = wf

        # ---- main compute ------------------------------------------------
        pre_img = None
        if PROBE == "mm_only":
            # PE-pure probe: one image pair loaded once, outside the loop
            pre_img = wp.tile([2 * CIN, HP, WP], mdt, tag="pre_img")
            nc.sync.dma_start(out=pre_img[:, :, :], in_=xin_d[0])

        def pack_branch(b, s, needs_log, psp):
            cfg = PACK_CFG[SCHEME]
            n_slots = cfg["n_slots"]
            wf = wf_tiles[s]
            if PROBE == "mm_only":
                img = pre_img
            else:
                img = imgp.tile([2 * CIN, HP, WP], mdt, tag="img")
            if PROBE not in ("no_in", "mm_only"):
                # one DMA for both partition halves: xin[b] is [2, CIN, HP, WP]
                nc.sync.dma_start(out=img[:, :, :], in_=xin_d[b])
                if needs_log:
                    nc.scalar.activation(
                        img[:, :, :],
                        img[:, :, :],
                        mybir.ActivationFunctionType.Ln,
                        bias=eps_b[:, 0:1],
                    )
            img2 = None
            if SCHEME == "pack5" and PROBE not in ("no_in", "no_in2", "mm_only"):
                # img2 = [A; C]: lower = A cols 0..63, upper = A cols 2..65
                img2 = img2p.tile([2 * CIN, HP, W], mdt, tag="img2")
                dq = nc.scalar if "d" in QCFG else nc.sync
                dq.dma_start(out=img2[0:CIN], in_=img[0:CIN, :, 0:W])
                dq.dma_start(out=img2[CIN : 2 * CIN], in_=img[0:CIN, :, 2 : 2 + W])
            elif SCHEME == "pack5h" and PROBE not in ("no_in", "no_in2", "mm_only"):
                img2 = img2p.tile([2 * CIN, HP, W], mdt, tag="img2")
                dq = nc.scalar if "d" in QCFG else nc.sync
                dq.dma_start(out=img2[:, :, :], in_=xin2_d[b])
                if needs_log:
                    nc.scalar.activation(
                        img2[:, :, :],
                        img2[:, :, :],
                        mybir.ActivationFunctionType.Ln,
                        bias=eps_b[:, 0:1],
                    )

            def rhs(slot, tp):
                t2, dr, c0 = cfg["rhs"][slot]
                r = 8 * tp + dr
                src = img2 if (t2 and img2 is not None) else img
                return src[:, r : r + 8, c0 : c0 + W]

            ps = [
                psp.tile([COUT, 8, W], F32, tag=f"ps{tp}", name=f"ps{tp}")
                for tp in range(8)
            ]
            if PROBE != "no_mm":
                if ORDER == "kout":
                    seq = [(slot, tp) for slot in range(n_slots) for tp in range(8)]
                else:
                    seq = [(slot, tp) for tp in range(8) for slot in range(n_slots)]
                for slot, tp in seq:
                    nc.tensor.matmul(
                        ps[tp][:, :, :],
                        wf[:, slot * COUT : (slot + 1) * COUT],
                        rhs(slot, tp),
                        start=(slot == 0),
                        stop=(slot == n_slots - 1),
                    )
            if PROBE in ("no_out", "mm_only"):
                return
            oq = nc.scalar if "o" in QCFG else nc.sync
            for tph in range(4):
                ot = obp.tile([COUT, 16, W], odt, tag="ot")
                for j in (0, 1):
                    tp = 2 * tph + j
                    dst = ot[:, 8 * j : 8 * j + 8, :]
                    if needs_log:
                        nc.scalar.activation(
                            dst,
                            ps[tp][:, :, :],
                            mybir.ActivationFunctionType.Exp,
                            bias=zero_b[:, 0:1],
                        )
                    elif j == 0:
                        nc.vector.tensor_copy(dst, ps[tp][:, :, :])
                    else:
                        nc.scalar.activation(
                            dst, ps[tp][:, :, :], mybir.ActivationFunctionType.Copy
                        )
                h0 = 16 * tph
                oq.dma_start(out=out_d[b, :, h0 : h0 + 16, :], in_=ot[:, :, :])

        def legacy_branch(b, s, needs_log, evac, psp):
            wf = wf_tiles[s]
            img = imgp.tile([2 * CIN, img_rows, WP], mdt, tag="img")
            nc.sync.dma_start(out=img[0:CIN], in_=xin_d[b, 0])
            nc.sync.dma_start(out=img[CIN : 2 * CIN], in_=xin_d[b, 1])
            if needs_log:
                nc.scalar.activation(
                    img[:, :, :],
                    img[:, :, :],
                    mybir.ActivationFunctionType.Ln,
                    bias=eps_b[:, 0:1],
                )
            for tp in range(8):
                ps = psp.tile([COUT, 8, W], F32, tag="ps")
                if PROBE != "no_mm":
                    if SCHEME == "dual":
                        _mm_dual(nc, ps, wf, img, tp)
                    else:
                        _mm_hsplit(nc, ps, wf, img, tp)
                if PROBE == "no_out":
                    continue
                ot = obp.tile([COUT, 8, W], odt, tag="ot")
                h0 = 8 * tp
                if needs_log:
                    nc.scalar.activation(
                        ot[:, :, :],
                        ps[:, :, :],
                        mybir.ActivationFunctionType.Exp,
                        bias=zero_b[:, 0:1],
                    )
                elif evac == "v":
                    nc.vector.tensor_copy(ot[:, :, :], ps[:, :, :])
                else:
                    nc.scalar.activation(
                        ot[:, :, :], ps[:, :, :], mybir.ActivationFunctionType.Copy
                    )
                nc.sync.dma_start(out=out_d[b, :, h0 : h0 + 8, :], in_=ot[:, :, :])

        def main_body(psp):
            for b, s, needs_log, evac in BRANCHES:
                if packed:
                    pack_branch(b, s, needs_log, psp)
                else:
                    legacy_branch(b, s, needs_log, evac, psp)

        with tc.tile_pool(name="psp", bufs=n_psum, space="PSUM") as psp:
            if LOOP_ITERS:
                with tc.For_i(0, LOOP_ITERS, 1):
                    main_body(psp)
            else:
                main_body(psp)


def _wfk(wf, k, half):
    p0, p1 = half * CIN, (half + 1) * CIN
    return wf[p0:p1, k * COUT : (k + 1) * COUT]


def _mm_dual(nc, ps, wf, img, tp):
    """out rows 8*tp..8*tp+7 from dual-copy image: partitions 0..63 hold the
    padded image A (rows 0..65), partitions 64..127 hold B with B[r]=A[r+2].

    6 matmuls per tile: 3x K=128 (offset pairs (-1,dw)+(+1,dw)), then the
    dh=0 row as K=64 matmuls -- (0,-1) on the lower row group packed with
    (0,+1) on the upper row group (concurrent), plus (0,0) on the lower."""
    h0 = 8 * tp
    n_mm = 6
    idx = [0]

    def step(lhsT, rhs):
        nc.tensor.matmul(
            ps[:, :, :], lhsT, rhs, start=(idx[0] == 0), stop=(idx[0] == n_mm - 1)
        )
        idx[0] += 1

    for dw in (-1, 0, 1):  # slots 0..2: K=128, lower k=dw+1, upper k=7+dw
        step(
            wf[:, (dw + 1) * COUT : (dw + 2) * COUT],
            img[:, h0 : h0 + 8, 1 + dw : 1 + dw + W],
        )
    # (0,-1) lower (slot3 low) ++ (0,+1) upper (slot3 high, B[h0-1]=A[h0+1])
    step(wf[0:CIN, 3 * COUT : 4 * COUT], img[0:CIN, h0 + 1 : h0 + 9, 0:W])
    if DH0_UPPER_PACK and tp > 0:
        step(
            wf[CIN : 2 * CIN, 3 * COUT : 4 * COUT],
            img[CIN : 2 * CIN, h0 - 1 : h0 + 7, 2 : 2 + W],
        )
    else:  # B row -1 unavailable (tp=0) or packing disabled: lower, slot 5
        step(wf[0:CIN, 5 * COUT : 6 * COUT], img[0:CIN, h0 + 1 : h0 + 9, 2 : 2 + W])
    # (0,0) lower (slot4 low)
    step(wf[0:CIN, 4 * COUT : 5 * COUT], img[0:CIN, h0 + 1 : h0 + 9, 1 : 1 + W])


def _mm_hsplit(nc, ps, wf, img, tp):
    """hsplit scheme: tile tp covers out rows 8*tp..+7; lower tiles (tp<4)
    read partitions 0..63, upper tiles read 64..127."""
    half = 0 if tp < 4 else 1
    p0, p1 = half * CIN, (half + 1) * CIN
    tpl = tp % 4
    for k in range(9):
        dh, dw = k // 3 - 1, k % 3 - 1
        r = 8 * tpl + 1 + dh + half  # lower: pad row - 0; upper: pad row - 31
        c0 = 1 + dw
        nc.tensor.matmul(
            ps[:, :, :],
            _wfk(wf, k, half),
            img[p0:p1, r : r + 8, c0 : c0 + W],
            start=(k == 0),
            stop=(k == 8),
        )


def build_program():
    key = (
        MM_DTYPE, SCHEME, LOOP_ITERS, DH0_UPPER_PACK, PROBE, LDW_OPT, OUT_DT,
        ORDER, QCFG,
    )
    if key in _PROG_CACHE:
        return _PROG_CACHE[key]
    img_rows = HS_ROWS if SCHEME == "hsplit" else HP
    nc = bacc.Bacc("TRN2", target_bir_lowering=False, debug=False)
    xin_d = nc.dram_tensor(
        "xin", [NBR, 2, CIN, img_rows, WP], _mm_dt(), kind="ExternalInput"
    ).ap()
    xin2_d = None
    if SCHEME == "pack5h":
        xin2_d = nc.dram_tensor(
            "xin2", [NBR, 2, CIN, HP, W], _mm_dt(), kind="ExternalInput"
        ).ap()
    w1x_d = nc.dram_tensor("w1x", [CIN, 9], F32, kind="ExternalInput").ap()
    w1y_d = nc.dram_tensor("w1y", [CIN, 9], F32, kind="ExternalInput").ap()
    w2xT_d = nc.dram_tensor("w2xT", [CIN, COUT], F32, kind="ExternalInput").ap()
    w2yT_d = nc.dram_tensor("w2yT", [CIN, COUT], F32, kind="ExternalInput").ap()
    out_d = nc.dram_tensor("out", [NBR, COUT, H, W], _out_dt(), kind="ExternalOutput").ap()
    with tile.TileContext(nc) as tc:
        _emit(nc, tc, xin_d, xin2_d, w1x_d, w1y_d, w2xT_d, w2yT_d, out_d)
    nc.compile()
    _PROG_CACHE[key] = nc
    return nc


def marshal_inputs(x, w1x, w1y, w2x, w2y):
    """Host-side data marshaling: shard over batch, zero-pad, build the
    per-partition-half copies for the selected scheme."""
    ndt = _np_in_dt()
    x = np.asarray(x, dtype=np.float32)
    xp = np.zeros((B, NBR, CIN, HP, WP), np.float32)
    xp[:, :, :, 1 : H + 1, 1 : W + 1] = x
    if SCHEME == "hsplit":
        xin = np.empty((B, NBR, 2, CIN, HS_ROWS, WP), ndt)
        xin[:, :, 0] = xp[:, :, :, 0:HS_ROWS, :].astype(ndt)
        xin[:, :, 1] = xp[:, :, :, HP - HS_ROWS : HP, :].astype(ndt)
    else:  # dual / pack6 / pack5: copies (A, B) with B[r] = A[r+2]
        xin = np.zeros((B, NBR, 2, CIN, HP, WP), ndt)
        xin[:, :, 0] = xp.astype(ndt)
        xin[:, :, 1, :, 0 : HP - 2, :] = xp[:, :, :, 2:HP, :].astype(ndt)
    w2xT = np.ascontiguousarray(np.asarray(w2x, np.float32).T)
    w2yT = np.ascontiguousarray(np.asarray(w2y, np.float32).T)
    w1x = np.ascontiguousarray(w1x, np.float32)
    w1y = np.ascontiguousarray(w1y, np.float32)
    maps = [
        {
            "xin": np.ascontiguousarray(xin[i]),
            "w1x": w1x,
            "w1y": w1y,
            "w2xT": w2xT,
            "w2yT": w2yT,
        }
        for i in range(B)
    ]
    if SCHEME == "pack5h":
        xin2 = np.empty((B, NBR, 2, CIN, HP, W), ndt)
        xin2[:, :, 0] = xp[:, :, :, :, 0:W].astype(ndt)
        xin2[:, :, 1] = xp[:, :, :, :, 2 : 2 + W].astype(ndt)
        for i in range(B):
            maps[i]["xin2"] = np.ascontiguousarray(xin2[i])
    return maps


def kernel(x, w1x, w1y, w2x, w2y):
    global LAST_EXEC_TIME_NS, LAST_RESULTS
    nc = build_program()
    in_maps = marshal_inputs(x, w1x, w1y, w2x, w2y)
    res = bass_utils.run_bass_kernel_spmd(
        nc, in_maps, list(range(N_CORES)), trace=TRACE
    )
    LAST_EXEC_TIME_NS = res.exec_time_ns
    LAST_RESULTS = res
    out = np.stack([res.results[i]["out"] for i in range(N_CORES)], axis=0)
    return np.asarray(out, np.float32)
